# revision 3
# baseline (speedup 1.0000x reference)
"""Trainium2 kernel for nn_Decoder_75935021793458.

Data-parallel over batch B=256 across 8 NeuronCores (32 batches/core).

Device computes the two heavy, fully-parallel pieces:
  1. Pointer-logit table  upre[b, j, s] = sum_h v[h] * tanh(cqq[b,j,h] + keys[b,s,h])
     (every possible previous-choice j): DVE broadcast-add -> ACT tanh ->
     PE fp32 matmuls accumulated into one [32, 4096] PSUM tile using
     one-hot-column lhsT matrices (batch b writes only PSUM partition b).
  2. low_logit dots lowd[b,j,n] = node_context[b,j,n,:] @ w_low over the full
     512 MB node_context stream: DVE mult + reduce per [128, 8, 128] tile.

Host does the tiny sequential 64-step sampling scan on [B, 64] tables with
bit-exact jax gumbel noise (the RNG is data-independent), plus the small
reward/log-prob reductions from original_data.
"""

import contextlib
import sys

import numpy as np

sys.path.insert(0, "/opt/trn_rl_repo")

import jax
import jax.numpy as jnp

import concourse.bacc as bacc
import concourse.mybir as mybir
from concourse import tile
from concourse.bass_utils import run_bass_kernel_spmd

B, S, N, E, H = 256, 64, 64, 128, 128
CCLIP = 10.0
NEG = 1e8
NCORES = 8
BS = B // NCORES  # 32 batches per core

F32 = mybir.dt.float32

_PROGRAM_CACHE = {}


def _build_program():
    if "nc" in _PROGRAM_CACHE:
        return _PROGRAM_CACHE["nc"]

    nc = bacc.Bacc(
        "TRN2",
        target_bir_lowering=False,
        debug=False,
        enable_asserts=False,
        num_devices=NCORES,
    )

    keysT = nc.dram_tensor("keysT", [BS, H, S], F32, kind="ExternalInput")
    cqqT = nc.dram_tensor("cqqT", [BS, H, S], F32, kind="ExternalInput")
    lhsv = nc.dram_tensor("lhsv", [H, BS * BS], F32, kind="ExternalInput")
    wrep = nc.dram_tensor("wrep", [128, H], F32, kind="ExternalInput")
    nodef = nc.dram_tensor("nodef", [BS * S * N, H], F32, kind="ExternalInput")
    upre = nc.dram_tensor("upre", [BS, S * S], F32, kind="ExternalOutput")
    lowd = nc.dram_tensor("lowd", [128, 128, 8], F32, kind="ExternalOutput")

    ctx = contextlib.ExitStack()
    with tile.TileContext(nc) as tc:
        cpool = ctx.enter_context(tc.tile_pool(name="const", bufs=1))
        wpool = ctx.enter_context(tc.tile_pool(name="work", bufs=3))
        npool = ctx.enter_context(tc.tile_pool(name="node", bufs=4))
        ppool = ctx.enter_context(tc.tile_pool(name="psum", bufs=1, space="PSUM"))

        wrep_t = cpool.tile([128, H], F32, tag="wrep")
        nc.sync.dma_start(out=wrep_t[:], in_=wrep.ap())
        lhsv_t = cpool.tile([H, BS * BS], F32, tag="lhsv")
        nc.sync.dma_start(out=lhsv_t[:], in_=lhsv.ap())

        ups = ppool.tile([BS, S * S], F32, tag="ups")  # full PSUM rows 0..31

        for b in range(BS):
            kt = wpool.tile([H, S], F32, tag="kt")
            nc.sync.dma_start(out=kt[:], in_=keysT.ap()[b])
            ct = wpool.tile([H, S], F32, tag="ct")
            nc.sync.dma_start(out=ct[:], in_=cqqT.ap()[b])

            at = wpool.tile([H, S * S], F32, tag="at")
            at3 = at[:].rearrange("p (j s) -> p j s", j=S)
            nc.vector.tensor_add(
                at3,
                ct[:].unsqueeze(2).to_broadcast([H, S, S]),
                kt[:].unsqueeze(1).to_broadcast([H, S, S]),
            )
            th = wpool.tile([H, S * S], F32, tag="th")
            nc.scalar.activation(th[:], at[:], mybir.ActivationFunctionType.Tanh)

            for c in range(8):
                nc.tensor.matmul(
                    ups[:, c * 512:(c + 1) * 512],
                    lhsT=lhsv_t[:, b * BS:(b + 1) * BS],
                    rhs=th[:, c * 512:(c + 1) * 512],
                    start=(b == 0),
                    stop=(b == BS - 1),
                )

            # interleave 4 node chunks per b so the 64MB stream overlaps
            for sub in range(4):
                start_row = b * (S * N) + sub * 1024
                nt = npool.tile([128, 8 * H], F32, tag="nt")
                src = nodef.ap()[start_row:start_row + 1024].rearrange(
                    "(m p) h -> p m h", p=128
                )
                nc.sync.dma_start(
                    out=nt[:].rearrange("p (m h) -> p m h", h=H), in_=src
                )
                pr = npool.tile([128, 8 * H], F32, tag="pr")
                nc.vector.tensor_mul(
                    pr[:].rearrange("p (m h) -> p m h", h=H),
                    nt[:].rearrange("p (m h) -> p m h", h=H),
                    wrep_t[:].unsqueeze(1).to_broadcast([128, 8, H]),
                )
                ac = npool.tile([128, 8], F32, tag="ac")
                nc.vector.tensor_reduce(
                    ac[:],
                    pr[:].rearrange("p (m h) -> p m h", h=H),
                    axis=mybir.AxisListType.X,
                    op=mybir.AluOpType.add,
                )
                nc.sync.dma_start(out=lowd.ap()[b * 4 + sub], in_=ac[:])

        ucp = cpool.tile([BS, S * S], F32, tag="ucp")
        nc.vector.tensor_copy(ucp[:], ups[:])
        nc.sync.dma_start(out=upre.ap(), in_=ucp[:])
        ctx.close()

    nc.compile()
    _PROGRAM_CACHE["nc"] = nc
    return nc


def _gumbel_all():
    """64 per-step gumbel draws, bit-identical to jax.random.categorical's."""
    if "G" in _PROGRAM_CACHE:
        return _PROGRAM_CACHE["G"]
    base = jax.random.key(42)
    try:
        dev = jax.devices("cpu")[0]
        cm = jax.default_device(dev)
    except Exception:
        cm = contextlib.nullcontext()
    with cm:
        G = np.stack(
            [
                np.asarray(
                    jax.random.gumbel(
                        jax.random.fold_in(base, i), (B, S), jnp.float32
                    )
                )
                for i in range(S)
            ]
        )
    _PROGRAM_CACHE["G"] = G
    return G


def kernel(
    node_context,
    original_data,
    cell_context,
    high_mask,
    low_mask,
    init_w,
    h_W,
    h_b,
    vw_W,
    vw_b,
    Wq,
    bq,
    Wk,
    bk,
    v,
    vb,
    w_low,
):
    f = np.float32
    node_context = np.asarray(node_context, f)
    original_data = np.asarray(original_data, f)
    cell = np.asarray(cell_context, f)
    high_mask = np.asarray(high_mask)
    low_mask = np.asarray(low_mask)
    init_w, h_W, h_b = np.asarray(init_w, f), np.asarray(h_W, f), np.asarray(h_b, f)
    vw_W, vw_b = np.asarray(vw_W, f), np.asarray(vw_b, f)
    Wq, bq, Wk, bk = (np.asarray(x, f) for x in (Wq, bq, Wk, bk))
    v, vb, w_low = np.asarray(v, f), np.asarray(vb, f), np.asarray(w_low, f)

    # ---- host: small linear algebra (exact fp32 mirror of the reference) ----
    h_bar = cell.mean(axis=1) @ h_W + h_b                       # [B,E]
    keys = cell @ Wk + bk                                       # [B,S,H]
    init_h = cell[:, 0, :]                                      # forced start cell
    # query for steps >=1 given prev choice j:
    #   q_j = (h_bar + init_h@vw_W[:E] + cell[:,j]@vw_W[E:] + vw_b) @ Wq + bq
    base_q = h_bar + init_h @ vw_W[:E] + vw_b                   # [B,E]
    q_j = base_q[:, None, :] + cell @ vw_W[E:]                  # [B,S,E]
    cqq = q_j @ Wq + bq                                         # [B,S,H]
    # step-0 logits (query0 path) on host: 1.5% of the tanh work
    query0 = h_bar + init_w @ vw_W + vw_b
    q0 = query0 @ Wq + bq                                       # [B,H]
    u0 = CCLIP * np.tanh(
        np.tanh(q0[:, None, :] + keys) @ v + vb
    ).astype(f)                                                 # [B,S]

    # ---- device: heavy parallel phases --------------------------------------
    nc = _build_program()
    lhsv_mats = np.zeros((H, BS * BS), f)
    for b in range(BS):
        lhsv_mats[:, b * BS + b] = v
    wrep_arr = np.ascontiguousarray(np.tile(w_low[None, :], (128, 1)))

    in_maps = []
    for c in range(NCORES):
        sl = slice(c * BS, (c + 1) * BS)
        in_maps.append(
            dict(
                keysT=np.ascontiguousarray(keys[sl].transpose(0, 2, 1)),
                cqqT=np.ascontiguousarray(cqq[sl].transpose(0, 2, 1)),
                lhsv=lhsv_mats,
                wrep=wrep_arr,
                nodef=np.ascontiguousarray(node_context[sl].reshape(-1, H)),
            )
        )
    res = run_bass_kernel_spmd(nc, in_maps, core_ids=list(range(NCORES)))
    kernel.last_results = res

    upre = np.concatenate([r["upre"] for r in res.results], axis=0)  # [B,4096]
    U = CCLIP * np.tanh(upre.reshape(B, S, S) + vb).astype(f)        # [B,j,s]
    lowd = np.concatenate(
        [r["lowd"].transpose(0, 2, 1).reshape(BS, S, N) for r in res.results],
        axis=0,
    )                                                                # [B,S,N]

    # ---- host: per-cell low-level stats ------------------------------------
    low_logits = lowd - NEG * low_mask.astype(f)                     # [B,S,N]
    m_low = low_logits.max(axis=-1)
    lse_low = m_low + np.log(
        np.exp(low_logits - m_low[..., None]).sum(axis=-1)
    )
    llp_cell = low_logits.sum(axis=-1) - N * lse_low                 # [B,S]
    lact_cell = np.argmax(low_logits, axis=-1).astype(np.int32)      # [B,S]
    d = np.diff(original_data, axis=2)
    lr_cell = np.sqrt((d * d).sum(-1)).sum(-1).astype(f)             # [B,S]
    entry = original_data[:, :, 0, :]                                # [B,S,2]
    exit_ = original_data[:, :, -1, :]                               # [B,S,2]

    # ---- host: sequential sampling scan ------------------------------------
    G = _gumbel_all()
    bidx = np.arange(B)
    mask = high_mask.astype(f).copy()
    hlp = np.zeros(B, f)
    llp = np.zeros(B, f)
    hr = np.zeros(B, f)
    lr = np.zeros(B, f)
    last_node = np.zeros((B, 2), f)
    hact = np.zeros((B, S), np.int32)
    lact = np.zeros((B, S), np.int32)
    idx = np.zeros(B, np.int64)
    for i in range(S):
        u_raw = u0 if i == 0 else U[bidx, idx]                       # [B,S]
        u_ = u_raw - NEG * mask
        m = u_.max(axis=-1)
        lse = m + np.log(np.exp(u_ - m[:, None]).sum(axis=-1))
        idx = np.argmax(u_ + G[i], axis=-1)
        if i == 0:
            idx = np.zeros(B, np.int64)
        hact[:, i] = idx
        hlp += (u_[bidx, idx] - lse).astype(f)
        llp += llp_cell[bidx, idx]
        lact[:, i] = lact_cell[bidx, idx]
        i_n = entry[bidx, idx]
        hr += np.sqrt(((last_node - i_n) ** 2).sum(-1)).astype(f)
        lr += lr_cell[bidx, idx]
        last_node = exit_[bidx, idx]
        oh = np.zeros((B, S), f)
        oh[bidx, idx] = 1.0
        mask = np.maximum(mask, oh)

    return (hlp, llp, hr, lr, hact, lact)


if __name__ == "__main__":
    _build_program()
    print("program built OK")


# revision 6
# speedup vs baseline: 1.0801x; 1.0801x over previous
"""Trainium2 kernel for nn_Decoder_75935021793458.

Data-parallel over batch B=256 across 8 NeuronCores (32 batches/core).

Device computes the two heavy, fully-parallel pieces:
  1. Pointer-logit table  upre[b, j, s] = sum_h v[h] * tanh(cqq[b,j,h] + keys[b,s,h])
     (every possible previous-choice j): DVE broadcast-add -> ACT tanh ->
     PE fp32 matmuls accumulated into one [32, 4096] PSUM tile using
     one-hot-column lhsT matrices (batch b writes only PSUM partition b).
  2. low_logit dots lowd[b,j,n] = node_context[b,j,n,:] @ w_low over the full
     512 MB node_context stream: DVE mult + reduce per [128, 8, 128] tile.

Host does the tiny sequential 64-step sampling scan on [B, 64] tables with
bit-exact jax gumbel noise (the RNG is data-independent), plus the small
reward/log-prob reductions from original_data.
"""

import contextlib
import sys

import numpy as np

sys.path.insert(0, "/opt/trn_rl_repo")

import jax
import jax.numpy as jnp

import concourse.bacc as bacc
import concourse.mybir as mybir
from concourse import tile
from concourse.bass_utils import run_bass_kernel_spmd

B, S, N, E, H = 256, 64, 64, 128, 128
CCLIP = 10.0
NEG = 1e8
NCORES = 8
BS = B // NCORES  # 32 batches per core

F32 = mybir.dt.float32

_PROGRAM_CACHE = {}


def _build_program():
    if "nc" in _PROGRAM_CACHE:
        return _PROGRAM_CACHE["nc"]

    nc = bacc.Bacc(
        "TRN2",
        target_bir_lowering=False,
        debug=False,
        enable_asserts=False,
        num_devices=NCORES,
    )

    keysT = nc.dram_tensor("keysT", [BS, H, S], F32, kind="ExternalInput")
    cqqT = nc.dram_tensor("cqqT", [BS, H, S], F32, kind="ExternalInput")
    lhsv = nc.dram_tensor("lhsv", [H, BS * BS], F32, kind="ExternalInput")
    wrep = nc.dram_tensor("wrep", [128, H], F32, kind="ExternalInput")
    nodef = nc.dram_tensor("nodef", [BS * S * N, H], F32, kind="ExternalInput")
    upre = nc.dram_tensor("upre", [BS, S * S], F32, kind="ExternalOutput")
    lowd = nc.dram_tensor("lowd", [128, 128, 8], F32, kind="ExternalOutput")

    ctx = contextlib.ExitStack()
    with tile.TileContext(nc) as tc:
        cpool = ctx.enter_context(tc.tile_pool(name="const", bufs=1))
        wpool = ctx.enter_context(tc.tile_pool(name="work", bufs=3))
        npool = ctx.enter_context(tc.tile_pool(name="node", bufs=6))
        ppool = ctx.enter_context(tc.tile_pool(name="psum", bufs=1, space="PSUM"))

        wrep_t = cpool.tile([128, H], F32, tag="wrep")
        nc.sync.dma_start(out=wrep_t[:], in_=wrep.ap())
        lhsv_t = cpool.tile([H, BS * BS], F32, tag="lhsv")
        nc.sync.dma_start(out=lhsv_t[:], in_=lhsv.ap())

        ups = ppool.tile([BS, S * S], F32, tag="ups")  # full PSUM rows 0..31

        for b in range(BS):
            kt = wpool.tile([H, S], F32, tag="kt")
            nc.sync.dma_start(out=kt[:], in_=keysT.ap()[b])
            ct = wpool.tile([H, S], F32, tag="ct")
            nc.sync.dma_start(out=ct[:], in_=cqqT.ap()[b])

            at = wpool.tile([H, S * S], F32, tag="at")
            at3 = at[:].rearrange("p (j s) -> p j s", j=S)
            nc.vector.tensor_add(
                at3,
                ct[:].unsqueeze(2).to_broadcast([H, S, S]),
                kt[:].unsqueeze(1).to_broadcast([H, S, S]),
            )
            th = wpool.tile([H, S * S], F32, tag="th")
            nc.scalar.activation(th[:], at[:], mybir.ActivationFunctionType.Tanh)

            for c in range(8):
                nc.tensor.matmul(
                    ups[:, c * 512:(c + 1) * 512],
                    lhsT=lhsv_t[:, b * BS:(b + 1) * BS],
                    rhs=th[:, c * 512:(c + 1) * 512],
                    start=(b == 0),
                    stop=(b == BS - 1),
                )

            # interleave 4 node chunks per b so the 64MB stream overlaps
            for sub in range(4):
                start_row = b * (S * N) + sub * 1024
                nt = npool.tile([128, 8 * H], F32, tag="nt")
                src = nodef.ap()[start_row:start_row + 1024].rearrange(
                    "(m p) h -> p m h", p=128
                )
                nc.sync.dma_start(
                    out=nt[:].rearrange("p (m h) -> p m h", h=H), in_=src
                )
                pr = npool.tile([128, 8 * H], F32, tag="pr")
                nc.vector.tensor_mul(
                    pr[:].rearrange("p (m h) -> p m h", h=H),
                    nt[:].rearrange("p (m h) -> p m h", h=H),
                    wrep_t[:].unsqueeze(1).to_broadcast([128, 8, H]),
                )
                ac = npool.tile([128, 8], F32, tag="ac")
                nc.vector.tensor_reduce(
                    ac[:],
                    pr[:].rearrange("p (m h) -> p m h", h=H),
                    axis=mybir.AxisListType.X,
                    op=mybir.AluOpType.add,
                )
                nc.sync.dma_start(out=lowd.ap()[b * 4 + sub], in_=ac[:])

        ucp = cpool.tile([BS, S * S], F32, tag="ucp")
        nc.vector.tensor_copy(ucp[:], ups[:])
        nc.sync.dma_start(out=upre.ap(), in_=ucp[:])
        ctx.close()

    nc.compile()
    _PROGRAM_CACHE["nc"] = nc
    return nc


def _gumbel_all():
    """64 per-step gumbel draws, bit-identical to jax.random.categorical's."""
    if "G" in _PROGRAM_CACHE:
        return _PROGRAM_CACHE["G"]
    base = jax.random.key(42)
    try:
        dev = jax.devices("cpu")[0]
        cm = jax.default_device(dev)
    except Exception:
        cm = contextlib.nullcontext()
    with cm:
        G = np.stack(
            [
                np.asarray(
                    jax.random.gumbel(
                        jax.random.fold_in(base, i), (B, S), jnp.float32
                    )
                )
                for i in range(S)
            ]
        )
    _PROGRAM_CACHE["G"] = G
    return G


def kernel(
    node_context,
    original_data,
    cell_context,
    high_mask,
    low_mask,
    init_w,
    h_W,
    h_b,
    vw_W,
    vw_b,
    Wq,
    bq,
    Wk,
    bk,
    v,
    vb,
    w_low,
):
    f = np.float32
    node_context = np.asarray(node_context, f)
    original_data = np.asarray(original_data, f)
    cell = np.asarray(cell_context, f)
    high_mask = np.asarray(high_mask)
    low_mask = np.asarray(low_mask)
    init_w, h_W, h_b = np.asarray(init_w, f), np.asarray(h_W, f), np.asarray(h_b, f)
    vw_W, vw_b = np.asarray(vw_W, f), np.asarray(vw_b, f)
    Wq, bq, Wk, bk = (np.asarray(x, f) for x in (Wq, bq, Wk, bk))
    v, vb, w_low = np.asarray(v, f), np.asarray(vb, f), np.asarray(w_low, f)

    # ---- host: small linear algebra (exact fp32 mirror of the reference) ----
    h_bar = cell.mean(axis=1) @ h_W + h_b                       # [B,E]
    keys = cell @ Wk + bk                                       # [B,S,H]
    init_h = cell[:, 0, :]                                      # forced start cell
    # query for steps >=1 given prev choice j:
    #   q_j = (h_bar + init_h@vw_W[:E] + cell[:,j]@vw_W[E:] + vw_b) @ Wq + bq
    base_q = h_bar + init_h @ vw_W[:E] + vw_b                   # [B,E]
    q_j = base_q[:, None, :] + cell @ vw_W[E:]                  # [B,S,E]
    cqq = q_j @ Wq + bq                                         # [B,S,H]
    # step-0 logits (query0 path) on host: 1.5% of the tanh work
    query0 = h_bar + init_w @ vw_W + vw_b
    q0 = query0 @ Wq + bq                                       # [B,H]
    u0 = CCLIP * np.tanh(
        np.tanh(q0[:, None, :] + keys) @ v + vb
    ).astype(f)                                                 # [B,S]

    # ---- device: heavy parallel phases --------------------------------------
    nc = _build_program()
    lhsv_mats = np.zeros((H, BS * BS), f)
    for b in range(BS):
        lhsv_mats[:, b * BS + b] = v
    wrep_arr = np.ascontiguousarray(np.tile(w_low[None, :], (128, 1)))

    in_maps = []
    for c in range(NCORES):
        sl = slice(c * BS, (c + 1) * BS)
        in_maps.append(
            dict(
                keysT=np.ascontiguousarray(keys[sl].transpose(0, 2, 1)),
                cqqT=np.ascontiguousarray(cqq[sl].transpose(0, 2, 1)),
                lhsv=lhsv_mats,
                wrep=wrep_arr,
                nodef=np.ascontiguousarray(node_context[sl].reshape(-1, H)),
            )
        )
    import os
    import time as _time

    _t0 = _time.time()
    res = run_bass_kernel_spmd(
        nc,
        in_maps,
        core_ids=list(range(NCORES)),
        trace=False,
    )
    kernel.last_spmd_ns = (_time.time() - _t0) * 1e9
    kernel.last_results = res

    upre = np.concatenate([r["upre"] for r in res.results], axis=0)  # [B,4096]
    U = CCLIP * np.tanh(upre.reshape(B, S, S) + vb).astype(f)        # [B,j,s]
    lowd = np.concatenate(
        [r["lowd"].transpose(0, 2, 1).reshape(BS, S, N) for r in res.results],
        axis=0,
    )                                                                # [B,S,N]

    # ---- host: per-cell low-level stats ------------------------------------
    low_logits = lowd - NEG * low_mask.astype(f)                     # [B,S,N]
    m_low = low_logits.max(axis=-1)
    lse_low = m_low + np.log(
        np.exp(low_logits - m_low[..., None]).sum(axis=-1)
    )
    llp_cell = low_logits.sum(axis=-1) - N * lse_low                 # [B,S]
    lact_cell = np.argmax(low_logits, axis=-1).astype(np.int32)      # [B,S]
    d = np.diff(original_data, axis=2)
    lr_cell = np.sqrt((d * d).sum(-1)).sum(-1).astype(f)             # [B,S]
    entry = original_data[:, :, 0, :]                                # [B,S,2]
    exit_ = original_data[:, :, -1, :]                               # [B,S,2]

    # ---- host: sequential sampling scan ------------------------------------
    G = _gumbel_all()
    bidx = np.arange(B)
    mask = high_mask.astype(f).copy()
    hlp = np.zeros(B, f)
    llp = np.zeros(B, f)
    hr = np.zeros(B, f)
    lr = np.zeros(B, f)
    last_node = np.zeros((B, 2), f)
    hact = np.zeros((B, S), np.int32)
    lact = np.zeros((B, S), np.int32)
    idx = np.zeros(B, np.int64)
    for i in range(S):
        u_raw = u0 if i == 0 else U[bidx, idx]                       # [B,S]
        u_ = u_raw - NEG * mask
        m = u_.max(axis=-1)
        lse = m + np.log(np.exp(u_ - m[:, None]).sum(axis=-1))
        idx = np.argmax(u_ + G[i], axis=-1)
        if i == 0:
            idx = np.zeros(B, np.int64)
        hact[:, i] = idx
        hlp += (u_[bidx, idx] - lse).astype(f)
        llp += llp_cell[bidx, idx]
        lact[:, i] = lact_cell[bidx, idx]
        i_n = entry[bidx, idx]
        hr += np.sqrt(((last_node - i_n) ** 2).sum(-1)).astype(f)
        lr += lr_cell[bidx, idx]
        last_node = exit_[bidx, idx]
        oh = np.zeros((B, S), f)
        oh[bidx, idx] = 1.0
        mask = np.maximum(mask, oh)

    return (hlp, llp, hr, lr, hact, lact)


if __name__ == "__main__":
    _build_program()
    print("program built OK")
